# revision 33
# baseline (speedup 1.0000x reference)
"""MeshConv (gnn_message_passing) Trainium2 kernel.

Math (per batch b):
    idx[e] = [e, ne0[e], ne1[e], ne2[e], ne3[e]]   (self + 4 neighbor edges)
    taps:  e0 = x[:, e],  ek = x[:, ne_{k-1}[e]]
    G = [e0, e1+e3, e2+e4, |e1-e3|, |e2-e4|]       (5 "taps" of 128 channels)
    out[o, e] = sum_{c,k} G[c, e, k] * W[o, c, k] + bias[o]

Strategy (8 NeuronCores):
  - Data parallel over (batch, edge-half): core = b*2 + h handles 15000 edges
    of batch b. Conv weight replicated.
  - Neighbor taps fetched with SWDGE dma_gather(transpose=True) from a
    host-pretransposed [E, 128] bf16 copy of x: each gather lands a
    [128 channels, N edges] bf16 tile directly in matmul-rhs layout.
  - Gather descriptor GENERATION is the kernel bottleneck (one gather runs on
    only 2 of 8 Q7 cores: pair (2q, 2q+1) for queue q). Gathers are issued as
    prepare_only on 4 SWDGE queues so 4 Q7 pairs generate concurrently (4x),
    while their DMA *transfers* are strictly serialized via per-queue
    trigger_dma chained with semaphore waits. Serialization is required for
    correctness: transpose-mode gather streams pass through the shared S2M
    xbar transpose unit, and two gathers' descriptor streams interleaving at
    the SDMA engines corrupt the gathered data (observed on HW; free-running
    multi-queue gave rel_err ~1.0 while CoreSim passed).
  - Tile's prepare_only support under TileContext wires consumers to wait on
    the prep's DMASW lane semaphore but bakes the user-provided `sem=` into
    the descriptors; _wire_swdge() rewrites each prep's DMA-completion update
    to its assigned DMASW lane semaphore so those waits actually fire, and
    appends the trigger-chain waits.
  - Tap combines (add / sub) on DVE in bf16, |.| on ACT.
  - 5 accumulating bf16 matmuls per 128-output-half into fp32 PSUM; bias is
    fused into the PSUM->SBUF copy on ACT (Identity activation with bias AP),
    output stored as bf16 (halves store traffic; host converts to f32).
"""

import os
import sys

sys.path.insert(0, "/opt/trn_rl_repo")

from contextlib import ExitStack

import ml_dtypes
import numpy as np

import concourse.bacc as bacc
import concourse.bass as bass
import concourse.bass_isa as bass_isa
import concourse.tile as tile
from concourse import mybir

BF16 = ml_dtypes.bfloat16

P = 128          # partitions / in-channels
B, C, E, KT = 4, 128, 30000, 5
CO = 256         # out-channels
NCORES = 8
EH = E // 2      # edges per core (half a batch)
TILE = 3072      # edges per gather macro-tile (multiple of 128); TWO gathers
                 # (2 x 194 descs/engine x 64B = 24.8KB) fit the enlarged 32KB
                 # SWDGE ring, so tile t+1's preps generate while t transfers
NT = 5           # macro-tiles per core; NT*TILE = 15360 >= EH
EPAD = TILE * NT
CH = 512         # matmul chunk: psum free dim (512 f32 = one 2KB bank)
NCH = TILE // CH
IDXW = TILE // 16  # idx entries per wrap column
NQ = 4           # SWDGE queues (ucode MAX_SWDGE_QUEUES)

_LAST_RESULTS = None  # BassKernelResults of the most recent run (for test.py)
_PROGRAM = None
# Combine-instruction name -> global gather index it must wait for. Tile's
# auto-sync drops the RAW edge for the SECOND input of a consumer of a
# prepare_only gather (observed: pt = g0+g2 waits only on g0's lane sem),
# so _wire_swdge rewrites these waits from this build-time record.
_COMBINE_NEEDS: dict[str, int] = {}


def _wire_swdge(nc: bass.Bass) -> None:
    """Fix up prepare_only gather semaphores and serialize the triggers.

    Tile's prepare_only contract leaves data synchronization to the user:
    its DMASW lane sems only track "prep issued" (IncSwdgeSem doorbell
    pre-bumps), while the SDMA engines bump the user sem= qsem at transfer
    completion. This pass (1) upgrades every DMASW lane wait to the
    equivalent qsem transfer-completion wait, (2) inserts a Pool-stream
    EventSemaphore before each trigger so at most one gather's descriptor
    stream is in flight at a time (shared-xbar safety), and (3) gives each
    tap combine its true data wait (Tile drops the in1 RAW edge).
    """
    import bass_rust

    from concourse.tile_scheduler import PROC_NAME_TO_IDX

    dmasw0 = PROC_NAME_TO_IDX["DMASW0"]
    insts = [i for blk in nc.m.functions[0].blocks for i in blk.instructions]

    lane_sems: dict[int, tuple[int, str]] = {}
    for i in insts:
        si = i.sync_info
        if si is None:
            continue
        for w in list(si.on_wait) + list(si.on_update):
            nm = w.ant_name or ""
            if nm.startswith("DMASW"):
                lane_sems[int(nm[5:].split("_")[0])] = (w.id, nm)

    preps = []
    pairs = []  # (trigger, its prep, block, trigger name) in final trigger order
    fifo: dict[int, list] = {q: [] for q in range(NQ)}
    blocks = list(nc.m.functions[0].blocks)
    for blk in blocks:
        for i in blk.instructions:
            if isinstance(i, mybir.InstDMAGatherAnt):
                assert i.gen_mode == 1, i.name
                fifo[i.queue_num].append(i)
                preps.append(i)
            elif isinstance(i, bass_isa.InstTriggerDma):
                q = i.queue_num
                assert fifo[q], f"trigger {i.name} has no pending prep on queue {q}"
                pairs.append((i, fifo[q].pop(0), blk))
    assert preps and all(not v for v in fifo.values()), (
        len(preps),
        {q: len(v) for q, v in fifo.items()},
    )
    # The global gather indices recorded at build time assume final order ==
    # build order (queues cycle 0..NQ-1).
    for gi, p in enumerate(preps):
        assert p.queue_num == gi % NQ, (gi, p.name, p.queue_num)

    def prep_lane(p):
        lane = p.bass_scheduled_proc - dmasw0
        assert 0 <= lane < 8, (p.name, p.bass_scheduled_proc)
        return lane

    # The SDMA engines bump the user qsem baked into each prep (sem= kwarg ->
    # sem_num -> sem-inc descriptors) by 16 at transfer completion. All data
    # synchronization added below uses those qsems; Tile's own machinery
    # (IncSwdgeSem doorbell pre-bumps, DMASW lane sems, ring bookkeeping) is
    # left completely untouched — its lane-sem waits model "prep issued" and
    # are merely satisfied early.
    def qsem_of(p):
        u0 = p.sync_info.on_update[0]
        assert u0.update_value == 16 and (u0.ant_name or "").startswith("qsem"), u0
        return u0.id, u0.ant_name

    def done_wait(g):
        """SyncWait: gather with global index g has fully transferred."""
        sid, snm = qsem_of(preps[g])
        return bass_rust.SyncWait(
            sync_type="semaphore",
            id=sid,
            ant_name=snm,
            wait_mode="sem-ge-imm",
            wait_value=16 * (g // NQ + 1),
            wait_reg=None,
        )

    # Upgrade every Tile-inserted DMASW lane wait (lane L >= 16k means "the
    # k-th prep on lane L has been ISSUED" — bumped by the IncSwdgeSem
    # doorbell pre-bump when the Pool stream passes the prep) to the
    # corresponding gather's transfer-completion qsem wait. These lane waits
    # guard WAR/WAW hazards (idx/x0 buffer reuse, gather-tile reuse) for
    # which issue-order is sufficient on HW but not in CoreSim's
    # deferred-to-trigger execution model; the completion wait is correct
    # for both and costs nothing extra given the serialized transfers.
    # IncSwdgeSem instructions keep their waits (doorbell protocol order).
    lane_tick_to_g = {
        (prep_lane(p), p.bass_scheduled_tick): g for g, p in enumerate(preps)
    }
    for blk in blocks:
        for i in blk.instructions:
            if isinstance(i, (bass_isa.InstIncSwdgeSem, bass_isa.InstTriggerDma)):
                # IncSwdgeSem: doorbell protocol order. TriggerDma: its DMASW
                # wait is desc-gen gating ("prep issued"), which is both
                # sufficient and non-circular — upgrading it to transfer
                # completion would make the trigger wait on the transfer it
                # itself fires.
                continue
            si = i.sync_info
            if si is None:
                continue
            for wi, w in enumerate(list(si.on_wait)):
                nm = w.ant_name or ""
                if not nm.startswith("DMASW"):
                    continue
                lane = int(nm[5:].split("_")[0])
                assert w.wait_value % 16 == 0, w
                g = lane_tick_to_g[(lane, w.wait_value // 16)]
                si.on_wait[wi] = done_wait(g)

    # Gate each trigger on its prep's Pool-engine completion (descriptors
    # committed to the ring). count=1 triggers carry only a nosync dep, so
    # Tile inserts no wait itself; compute the Pool engine sem's cumulative
    # value at each prep from the final stream and set the trigger's single
    # wait slot to it.
    pool_sid, pool_snm = None, None
    for p in preps:
        for u in p.sync_info.on_update:
            if (u.ant_name or "").startswith("Pool"):
                pool_sid, pool_snm = u.id, u.ant_name
        break
    assert pool_sid is not None
    pool_cum = 0
    prep_pool_val = {}
    for blk in blocks:
        for i in blk.instructions:
            si = i.sync_info
            if si is None:
                continue
            for u in si.on_update:
                if u.id == pool_sid:
                    pool_cum += u.update_value
            if isinstance(i, mybir.InstDMAGatherAnt):
                prep_pool_val[i.name] = pool_cum
    import bass_rust as _br
    for trig, prep, _ in pairs:
        if trig.sync_info is None:
            trig.sync_info = _br.SyncInfo(on_wait=[], on_update=[])
        si = trig.sync_info
        while si.on_wait:
            si.on_wait.pop()
        si.on_wait.append(
            bass_rust.SyncWait(
                sync_type="semaphore",
                id=pool_sid,
                ant_name=pool_snm,
                wait_mode="sem-ge-imm",
                wait_value=prep_pool_val[prep.name],
                wait_reg=None,
            )
        )

    # Serialize the gather transfers: before trigger j, insert a Pool-stream
    # EventSemaphore that waits for gather j-1's transfer to complete. (The
    # wait cannot live on the trigger itself: InstTriggerDma's codegen
    # accepts only one sync wait, already used for desc-gen gating.)
    for j, (trig, _, blk) in enumerate(pairs[1:], start=1):
        ev = mybir.InstEventSemaphore(name=f"xbar-chain-{j}", ins=[], outs=[])
        ev.engine = mybir.EngineType.Pool
        ev.sync_info = bass_rust.SyncInfo(on_wait=[done_wait(j - 1)], on_update=[])
        pos = blk.instructions.index(trig)
        blk.instructions.insert(pos, ev)

    # Give each tap-combine a true data wait (Tile's lane-sem waits only
    # order against "prep issued", and it drops the in1 RAW edge entirely).
    # Because transfers complete in strict global order, waiting for gather
    # g's qsem also covers every gather < g. Engine instructions accept only
    # ONE sync wait in codegen, so the combine's pre-existing waits are
    # spilled onto an EventSemaphore inserted just before it in its engine
    # stream.
    fixed = 0
    for blk in blocks:
        for i in list(blk.instructions):
            gi = _COMBINE_NEEDS.get(i.name)
            if gi is None:
                continue
            si = i.sync_info
            kept = list(si.on_wait)
            while si.on_wait:
                si.on_wait.pop()
            if kept:
                ev = mybir.InstEventSemaphore(
                    name=f"comb-wait-{fixed}", ins=[], outs=[]
                )
                ev.engine = i.engine
                ev.sync_info = bass_rust.SyncInfo(on_wait=kept, on_update=[])
                blk.instructions.insert(blk.instructions.index(i), ev)
            si.on_wait.append(done_wait(gi))
            fixed += 1
    assert fixed == len(_COMBINE_NEEDS), (fixed, len(_COMBINE_NEEDS))


def build_program(nt: int = NT) -> bass.Bass:
    _COMBINE_NEEDS.clear()
    # detect_race_conditions=False: the CoreSim race detector pairs each read
    # wait with the writer's sem update directly and cannot see the
    # transitive ordering DMASW2>=16 => DMASW0 done that the serialized
    # trigger chain guarantees; values are still checked by test.py.
    nc = bacc.Bacc(
        "TRN2",
        num_swdge_queues=NQ,
        detect_race_conditions=False,
        dynamic_dma_scratch_size=32768,
    )
    xt = nc.declare_dram_parameter("xt", [E, C], mybir.dt.bfloat16, isOutput=False)
    x0 = nc.declare_dram_parameter("x0", [C, EPAD], mybir.dt.bfloat16, isOutput=False)
    idx = nc.declare_dram_parameter(
        "idx", [NT, P, 4 * IDXW], mybir.dt.int16, isOutput=False
    )
    wt = nc.declare_dram_parameter("wt", [P, KT * CO], mybir.dt.bfloat16, isOutput=False)
    bias = nc.declare_dram_parameter("bias", [P, 2], mybir.dt.float32, isOutput=False)
    out = nc.declare_dram_parameter("out", [CO, EH], mybir.dt.bfloat16, isOutput=True)

    with tile.TileContext(nc) as tc, ExitStack() as ctx:
        consts = ctx.enter_context(tc.tile_pool(name="consts", bufs=1))
        gpool = ctx.enter_context(tc.tile_pool(name="gath", bufs=2))
        cpool = ctx.enter_context(tc.tile_pool(name="comb", bufs=2))
        opool = ctx.enter_context(tc.tile_pool(name="outs", bufs=4))
        psum = ctx.enter_context(tc.tile_pool(name="psum", bufs=3, space="PSUM"))

        from concourse.instruction_name_ordered_set import InstructionNameOrderedSet

        qsem = [nc.alloc_semaphore(f"qsem{q}") for q in range(NQ)]

        def emit_triggers(prep_names):
            # count=1 fires the oldest untriggered ring entry on the queue
            # (= the prep named here, by construction). The explicit sync
            # dep reproduces count=None's gating: Tile turns it into a wait
            # on the prep's Pool-engine completion (descriptors committed).
            for k, nm in enumerate(prep_names):
                tr = nc.gpsimd.trigger_dma(count=1, queue_num=k)
                deps = InstructionNameOrderedSet()
                deps.add(nm)
                tr.ins.add_nosync_dependencies_from(deps)

        wt_t = consts.tile([P, KT * CO], mybir.dt.bfloat16)
        nc.sync.dma_start(out=wt_t[:], in_=wt[:])
        bias_t = consts.tile([P, 2], mybir.dt.float32)
        nc.sync.dma_start(out=bias_t[:], in_=bias[:])

        def load_tile_inputs(t):
            it = gpool.tile([P, 4 * IDXW], mybir.dt.int16, tag="idx")
            nc.sync.dma_start(out=it[:], in_=idx[t])
            xt0 = gpool.tile([P, TILE], mybir.dt.bfloat16, tag="x0")
            nc.sync.dma_start(out=xt0[:], in_=x0[:, t * TILE : (t + 1) * TILE])
            return it, xt0

        prev_preps = None
        nxt = load_tile_inputs(0)
        for t in range(nt):
            idx_t, x0_t = nxt

            g = []
            tile_preps = []
            for k in range(4):
                gk = gpool.tile([P, TILE], mybir.dt.bfloat16, tag=f"g{k}")
                pr = nc.gpsimd.dma_gather(
                    gk[:].rearrange("p (a n) -> p a n", a=1),
                    xt[:],
                    idx_t[:, k * IDXW : (k + 1) * IDXW],
                    num_idxs=TILE,
                    num_idxs_reg=TILE,
                    elem_size=C,
                    transpose=True,
                    # single_packet=True corrupts data for >512-descriptor
                    # gathers (exceeds the SDMA packet limits) — multi-packet
                    # mode is required for correctness at this size.
                    single_packet=False,
                    queue_num=k,
                    prepare_only=True,
                    sem=qsem[k],
                )
                tile_preps.append(pr.ins.name)
                g.append(gk)
            # Software pipeline: the PREVIOUS tile's triggers are emitted
            # after THIS tile's preps, so this tile's descriptor generation
            # (on the 4 Q7 pairs) overlaps the previous tile's serialized
            # transfer chain instead of waiting behind it in the in-order
            # Pool stream. The next tile's idx/x0 loads are emitted here too
            # so they precede this tile's out-stores in the SP stream —
            # loads(t+1) behind stores(t) would deadlock: stores(t) need
            # triggers(t), which sit after preps(t+1), which need idx(t+1).
            if t + 1 < nt:
                nxt = load_tile_inputs(t + 1)
            if prev_preps is not None:
                emit_triggers(prev_preps)
            prev_preps = tile_preps

            pt = cpool.tile([P, TILE], mybir.dt.bfloat16, tag="p")
            ci_ = nc.vector.tensor_tensor(
                out=pt[:], in0=g[0][:], in1=g[2][:], op=mybir.AluOpType.add
            )
            _COMBINE_NEEDS[ci_.ins.name] = 4 * t + 2
            qt = cpool.tile([P, TILE], mybir.dt.bfloat16, tag="q")
            ci_ = nc.vector.tensor_tensor(
                out=qt[:], in0=g[1][:], in1=g[3][:], op=mybir.AluOpType.add
            )
            _COMBINE_NEEDS[ci_.ins.name] = 4 * t + 3
            d13 = cpool.tile([P, TILE], mybir.dt.bfloat16, tag="d13")
            ci_ = nc.vector.tensor_tensor(
                out=d13[:], in0=g[0][:], in1=g[2][:], op=mybir.AluOpType.subtract
            )
            _COMBINE_NEEDS[ci_.ins.name] = 4 * t + 2
            d24 = cpool.tile([P, TILE], mybir.dt.bfloat16, tag="d24")
            ci_ = nc.vector.tensor_tensor(
                out=d24[:], in0=g[1][:], in1=g[3][:], op=mybir.AluOpType.subtract
            )
            _COMBINE_NEEDS[ci_.ins.name] = 4 * t + 3
            a13 = cpool.tile([P, TILE], mybir.dt.bfloat16, tag="a13")
            nc.scalar.activation(
                out=a13[:], in_=d13[:], func=mybir.ActivationFunctionType.Abs
            )
            a24 = cpool.tile([P, TILE], mybir.dt.bfloat16, tag="a24")
            nc.scalar.activation(
                out=a24[:], in_=d24[:], func=mybir.ActivationFunctionType.Abs
            )

            rhs = [x0_t, pt, qt, a13, a24]
            for ci in range(NCH):
                col0 = t * TILE + ci * CH
                ncols = min(CH, EH - col0)
                if ncols <= 0:
                    break
                for h in range(2):
                    ps = psum.tile([P, CH], mybir.dt.float32, tag=f"ps{h}")
                    for k in range(KT):
                        nc.tensor.matmul(
                            out=ps[:],
                            lhsT=wt_t[:, k * CO + h * P : k * CO + h * P + P],
                            rhs=rhs[k][:, ci * CH : (ci + 1) * CH],
                            start=(k == 0),
                            stop=(k == KT - 1),
                        )
                    ot = opool.tile([P, CH], mybir.dt.bfloat16, tag=f"o{h}")
                    nc.scalar.activation(
                        out=ot[:],
                        in_=ps[:],
                        func=mybir.ActivationFunctionType.Identity,
                        bias=bias_t[:, h : h + 1],
                    )
                    nc.sync.dma_start(
                        out=out[h * P : (h + 1) * P, col0 : col0 + ncols],
                        in_=ot[:, :ncols],
                    )
        emit_triggers(prev_preps)
    nc.finalize()
    _wire_swdge(nc)
    return nc


def make_in_maps(x, ne_idx, conv_w, conv_b):
    xs = np.asarray(x)[..., 0]  # [B, C, E] f32
    xtb = np.ascontiguousarray(xs.transpose(0, 2, 1)).astype(BF16)  # [B, E, C]
    x0b = xs.astype(BF16)  # [B, C, E]

    wt_host = np.zeros((P, KT * CO), np.float32)
    for k in range(KT):
        wt_host[:, k * CO : (k + 1) * CO] = conv_w[:, :, 0, k].T
    wt_host = wt_host.astype(BF16)
    bias_host = np.ascontiguousarray(np.asarray(conv_b).reshape(2, P).T).astype(
        np.float32
    )

    in_maps = []
    for core in range(NCORES):
        b, h = divmod(core, 2)
        lo = h * EH
        x0c = np.zeros((C, EPAD), BF16)
        x0c[:, :EH] = x0b[b][:, lo : lo + EH]
        idxc = np.zeros((EPAD, 4), np.int16)
        idxc[:EH] = np.asarray(ne_idx)[b, lo : lo + EH, :].astype(np.int16)
        wrapped = idxc.reshape(NT, IDXW, 16, 4).transpose(0, 2, 3, 1)  # [t,p16,k,s]
        rep = np.broadcast_to(wrapped[:, None], (NT, 8, 16, 4, IDXW)).reshape(
            NT, P, 4 * IDXW
        )
        in_maps.append(
            {
                "xt": xtb[b],
                "x0": x0c,
                "idx": np.ascontiguousarray(rep),
                "wt": wt_host,
                "bias": bias_host,
            }
        )
    return in_maps


def kernel(x, ne_idx, conv_w, conv_b):
    global _LAST_RESULTS, _PROGRAM
    from concourse.bass_utils import run_bass_kernel_spmd

    in_maps = make_in_maps(x, ne_idx, conv_w, conv_b)
    if _PROGRAM is None:
        _PROGRAM = build_program()
    res = run_bass_kernel_spmd(
        _PROGRAM,
        in_maps,
        core_ids=list(range(NCORES)),
        trace=bool(os.environ.get("KERNEL_TRACE")),
    )
    _LAST_RESULTS = res

    out_full = np.zeros((B, CO, E), np.float32)
    for core in range(NCORES):
        b, h = divmod(core, 2)
        out_full[b, :, h * EH : (h + 1) * EH] = res.results[core]["out"].astype(
            np.float32
        )
    return out_full[..., None]
